# revision 1
# baseline (speedup 1.0000x reference)
"""AFT (attention-free transformer) block kernel for 8 Trainium2 NeuronCores.

Sharding: batch b in 0..3 -> core pair (2b, 2b+1); each core handles 4096
contiguous tokens of that batch's 8192-token sequence.  The only cross-core
dependency is the cumsum carry (per-channel totals of w=exp(k_norm) and
kv=w*v over the first half), exchanged with a per-pair AllGather; odd cores
apply the received carry, even cores multiply it by a 0 mask.

Layout: everything is [token=partition, channel=free].  Matmuls run in bf16
(inputs pre-transposed).  The per-128-token cumsum is a lower-triangular
matmul on the PE; the running carry stays fp32 and already broadcast across
partitions: an all-ones matmul yields the tile's column-sum replicated on
every partition, so the carry update is a single vector add per tile.
"""

import sys
import numpy as np
import ml_dtypes

for _p in ("/opt/trn_rl_repo",):
    if _p not in sys.path:
        sys.path.insert(0, _p)

P = 128
D = 1024
N_CORES = 8
B_FULL, T_FULL = 4, 8192
CHUNK = T_FULL // 2          # tokens per core
NT_FULL = CHUNK // P         # 32 tiles per core
RMS_EPS = 1.1920929e-07
AFT_EPS = 1e-6

_nc_cache = {}
USE_FP8 = False

_ACT_TABLES_PATCHED = False


def _restrict_act_tables():
    # Confine activation-table choice to two sets (phase A: ln/exp/square,
    # phase B: sigmoid) so the ACT engine loads each table once instead of
    # thrashing between per-function tables. Ids (dict order) are preserved;
    # emptied sets are merely unchoosable.
    global _ACT_TABLES_PATCHED
    if _ACT_TABLES_PATCHED:
        return
    import concourse.bacc as bacc_mod

    keep = {"natural_log_exp_and_others", "sigmoid_and_others"}
    orig = bacc_mod.get_activation_tables

    def restricted(arch, _orig=orig, _keep=keep):
        return {
            name: (funcs if name in _keep else set())
            for name, funcs in _orig(arch).items()
        }

    bacc_mod.get_activation_tables = restricted
    _ACT_TABLES_PATCHED = True


def build_nc(n_tiles=NT_FULL, num_devices=N_CORES, use_collective=True, use_fp8=True):
    import concourse.mybir as mybir
    import concourse.tile as tile
    from concourse import bacc

    AF = mybir.ActivationFunctionType
    fp32 = mybir.dt.float32
    bf16 = mybir.dt.bfloat16
    f8 = mybir.dt.float8e4 if use_fp8 else mybir.dt.bfloat16
    DR = mybir.MatmulPerfMode.DoubleRow if use_fp8 else None
    chunk = n_tiles * P

    _restrict_act_tables()
    nc = bacc.Bacc(
        "TRN2",
        target_bir_lowering=False,
        debug=False,
        enable_asserts=False,
        num_devices=num_devices,
    )

    x_d = nc.dram_tensor("x", [chunk, D], fp32, kind="ExternalInput")
    wqkv_d = nc.dram_tensor("wqkvT", [D, 3 * D], f8, kind="ExternalInput")
    wsw_d = nc.dram_tensor("wswT", [D, 2 * D], f8, kind="ExternalInput")
    wout_d = nc.dram_tensor("woutT", [D, D], f8, kind="ExternalInput")
    tri_d = nc.dram_tensor("triT", [P, P], bf16, kind="ExternalInput")
    mask_d = nc.dram_tensor("cmask", [2, 1], fp32, kind="ExternalInput")
    out_d = nc.dram_tensor("out", [chunk, D], fp32, kind="ExternalOutput")

    x_t = x_d.ap().rearrange("(n p) d -> n p d", p=P)
    out_t = out_d.ap().rearrange("(n p) d -> n p d", p=P)

    H = D // 2  # 512, matmul free-dim chunk

    with tile.TileContext(nc) as tc:
        with (
            tc.tile_pool(name="consts", bufs=1) as consts,
            tc.tile_pool(name="wbl", bufs=3) as wbl,
            tc.tile_pool(name="dram", bufs=1, space="DRAM") as dram,
        ):
            # ---- persistent constants in SBUF ----
            tri_sb = consts.tile([P, P], bf16)
            nc.sync.dma_start(tri_sb[:], tri_d.ap())
            ones_sb = consts.tile([P, P], bf16)
            nc.any.memset(ones_sb[:], 1.0)
            mask_sb = consts.tile([2, 1], fp32)
            nc.sync.dma_start(mask_sb[:], mask_d.ap())
            eps_sb = consts.tile([P, 1], fp32)
            nc.any.memset(eps_sb[:], RMS_EPS)

            # ---- DRAM scratch for phase A -> B ----
            wcum_dram = dram.tile([n_tiles, P, D], bf16)
            kvcum_dram = dram.tile([n_tiles, P, D], bf16)
            sigq_dram = dram.tile([n_tiles, P, D], bf16)
            cc_in = dram.tile([2, D], fp32)
            cc_out = dram.tile([4, D], fp32)

            # phase-B weights get a dedicated pool that coexists with phase A
            # so their SWDGE loads overlap phase A instead of waiting on a
            # WAR-reused SBUF range at the phase boundary.
            wsw_sb = consts.tile([P, 8, 2 * D], f8)
            wout_sb = consts.tile([P, 8, D], f8)

            # =========================== PHASE A ===========================
            with (
                tc.tile_pool(name="ps_qkv", bufs=4, space="PSUM") as ps_qkv,
                tc.tile_pool(name="ps_scan", bufs=2, space="PSUM") as ps_scan,
                tc.tile_pool(name="wka", bufs=3) as wk,
                tc.tile_pool(name="cbp", bufs=2) as cbp,
                tc.tile_pool(name="wqa", bufs=1) as wqa,
            ):
                # prefetch tile 0's x ahead of the weight streams
                xt0 = wk.tile([P, D], fp32, tag="xt", bufs=2, name="xt0")
                nc.sync.dma_start(xt0[:], x_t[0])

                wq_ap = wqkv_d.ap().rearrange("(ko p) n -> p ko n", p=P)
                wqkv_ks = []
                for kk in range(8):
                    wq_k = wqa.tile([P, 3 * D], f8, name=f"wq_k{kk}")
                    nc.gpsimd.dma_start(wq_k[:], wq_ap[:, kk, :])
                    wqkv_ks.append(wq_k)
                wsw_ap = wsw_d.ap().rearrange("(ko p) n -> p ko n", p=P)
                wout_ap = wout_d.ap().rearrange("(ko p) n -> p ko n", p=P)
                bweight_dmas = []
                for kk in range(8):
                    bweight_dmas.append(
                        nc.gpsimd.dma_start(wsw_sb[:, kk, :], wsw_ap[:, kk, :])
                    )
                    bweight_dmas.append(
                        nc.gpsimd.dma_start(wout_sb[:, kk, :], wout_ap[:, kk, :])
                    )

                # running carry (already broadcast to 128 partitions); starts 0
                cb = {}
                for t in ("w", "kv"):
                    cb[t] = cbp.tile([P, D], fp32, tag=f"cb_{t}", name=f"cb_{t}")
                    nc.any.memzero(cb[t][:])

                for i in range(n_tiles):
                    if i == 0:
                        xt = xt0
                    else:
                        xt = wk.tile([P, D], fp32, tag="xt", bufs=2)
                        nc.sync.dma_start(xt[:], x_t[i])

                    # rms_norm(x)
                    sq = wk.tile([P, D], fp32, tag="sqscratch", bufs=2)
                    ssq = wk.tile([P, 1], fp32, tag="ssq")
                    nc.scalar.activation(sq[:], xt[:], AF.Square, accum_out=ssq[:])
                    # rsqrt via exp(-0.5*ln(mean+eps)): stays in the ln/exp table
                    lms = wk.tile([P, 1], fp32, tag="lms")
                    nc.scalar.activation(
                        lms[:], ssq[:], AF.Ln, scale=1.0 / D, bias=eps_sb[:]
                    )
                    rs = wk.tile([P, 1], fp32, tag="rs")
                    nc.scalar.activation(rs[:], lms[:], AF.Exp, scale=-0.5)
                    xn = wk.tile([P, D], bf16, tag="xn")
                    nc.vector.tensor_scalar_mul(xn[:], xt[:], rs[:])

                    # transpose xn for matmul lhsT (single xbar DMA)
                    xnT = wk.tile([P, 8, P], bf16, tag="xnT", bufs=2)
                    nc.sync.dma_start_transpose(xnT[:], xn[:])
                    if use_fp8:
                        xnT8 = wk.tile([P, 8, P], f8, tag="xnT8", bufs=2)
                        nc.scalar.copy(xnT8[:], xnT[:])
                    else:
                        xnT8 = xnT

                    # qkv chunk-pair matmul: chunks (c0, c0+1) of 6x512
                    def mm_pair(c0):
                        pair = [
                            ps_qkv.tile([P, H], fp32, tag="qkv", name=f"qkv{c0}_{c}")
                            for c in range(2)
                        ]
                        for m in range(8):
                            for ci in range(2):
                                nc.tensor.matmul(
                                    pair[ci][:],
                                    lhsT=xnT8[:, m, :],
                                    rhs=wqkv_ks[m][
                                        :, (c0 + ci) * H : (c0 + ci + 1) * H
                                    ],
                                    start=(m == 0),
                                    stop=(m == 7),
                                    perf_mode=None,
                                )
                        return pair

                    def rms_scale(pair, nm):
                        sq2 = wk.tile([P, H], fp32, tag="sqscratch", name=f"sq2_{nm}", bufs=2)
                        pa = wk.tile([P, 1], fp32, tag=f"pa_{nm}", name=f"pa_{nm}")
                        pb = wk.tile([P, 1], fp32, tag=f"pb_{nm}", name=f"pb_{nm}")
                        nc.scalar.activation(
                            sq2[:], pair[0][:], AF.Square, accum_out=pa[:]
                        )
                        nc.scalar.activation(
                            sq2[:], pair[1][:], AF.Square, accum_out=pb[:]
                        )
                        st = wk.tile([P, 1], fp32, tag=f"st_{nm}", name=f"st_{nm}")
                        nc.vector.tensor_add(st[:], pa[:], pb[:])
                        nc.scalar.activation(
                            st[:], st[:], AF.Ln, scale=1.0 / D, bias=eps_sb[:]
                        )
                        rr = wk.tile([P, 1], fp32, tag=f"rr_{nm}", name=f"rr_{nm}")
                        nc.scalar.activation(rr[:], st[:], AF.Exp, scale=-0.5)
                        return rr

                    # k chunks -> w = exp(rms(k))
                    kp = mm_pair(2)
                    rsk = rms_scale(kp, "k")
                    w_sb = wk.tile([P, D], bf16, tag="w_sb", bufs=2)
                    for j in range(2):
                        js = slice(j * H, (j + 1) * H)
                        nc.scalar.activation(
                            w_sb[:, js], kp[j][:], AF.Exp, scale=rsk[:]
                        )

                    # q chunks -> rms(q) spilled (sigmoid applied in phase B)
                    qp = mm_pair(0)
                    rsq = rms_scale(qp, "q")
                    sigq = wk.tile([P, D], bf16, tag="sigq", bufs=2)
                    for j in range(2):
                        js = slice(j * H, (j + 1) * H)
                        nc.vector.tensor_scalar_mul(sigq[:, js], qp[j][:], rsq[:])
                    nc.sync.dma_start(sigq_dram[i], sigq[:])

                    # v chunks -> kv = w * v
                    vp = mm_pair(4)
                    kv_sb = wk.tile([P, D], bf16, tag="kv_sb", bufs=2)
                    for j in range(2):
                        js = slice(j * H, (j + 1) * H)
                        nc.vector.tensor_mul(kv_sb[:, js], w_sb[:, js], vp[j][:])

                    # chunked causal cumsum + broadcast carry chain
                    for t, src, dst in (
                        ("w", w_sb, wcum_dram),
                        ("kv", kv_sb, kvcum_dram),
                    ):
                        cum = wk.tile([P, D], bf16, tag=f"cum_{t}", name=f"cum_{t}", bufs=2)
                        ps = ps_scan.tile([P, D], fp32, tag="scan", name=f"scan_{t}")
                        for j in range(2):
                            js = slice(j * H, (j + 1) * H)
                            nc.tensor.matmul(
                                ps[:, js], lhsT=tri_sb[:], rhs=src[:, js],
                                start=True, stop=True,
                            )
                        nc.vector.tensor_add(cum[:], ps[:], cb[t][:])
                        nc.sync.dma_start(dst[i], cum[:])
                        nxt = cbp.tile([P, D], fp32, tag=f"cb_{t}", name=f"cbn_{t}")
                        ps2 = ps_scan.tile([P, D], fp32, tag="scan", name=f"col_{t}")
                        for j in range(2):
                            js = slice(j * H, (j + 1) * H)
                            nc.tensor.matmul(
                                ps2[:, js], lhsT=ones_sb[:], rhs=src[:, js],
                                start=True, stop=True,
                            )
                        nc.vector.tensor_add(nxt[:], cb[t][:], ps2[:])
                        if i + 1 < n_tiles:
                            cb[t] = nxt
                        else:
                            row = 0 if t == "w" else 1
                            nc.sync.dma_start(cc_in[row : row + 1, :], nxt[0:1, :])

            # ======================= carry exchange ========================
            import concourse.mybir as _mybir

            gath = consts.tile([2, D], fp32)
            if use_collective:
                nc.gpsimd.collective_compute(
                    "AllGather",
                    _mybir.AluOpType.bypass,
                    replica_groups=[[2 * p, 2 * p + 1] for p in range(num_devices // 2)],
                    ins=[cc_in[:].opt()],
                    outs=[cc_out[:].opt()],
                    cc_dim="Partition",
                )
                nc.sync.dma_start(gath[:], cc_out[0:2, :])
            else:
                nc.any.memzero(gath[:])

            gathm = consts.tile([2, D], fp32)
            nc.vector.tensor_scalar_mul(gathm[:], gath[:], mask_sb[:])
            row1 = consts.tile([1, D], fp32)
            nc.sync.dma_start(row1[:], gathm[1:2, :])
            cwb32 = consts.tile([P, D], fp32)
            ckb32 = consts.tile([P, D], fp32)
            nc.gpsimd.partition_broadcast(cwb32[:], gathm[0:1, :])
            nc.gpsimd.partition_broadcast(ckb32[:], row1[:])
            cwb = consts.tile([P, D], bf16)
            ckb = consts.tile([P, D], bf16)
            # fold the 1e-6 denominator epsilon into the w-carry tile
            nc.vector.tensor_scalar_add(cwb[:], cwb32[:], AFT_EPS)
            nc.vector.tensor_copy(ckb[:], ckb32[:])

            # =========================== PHASE B ===========================
            with (
                tc.tile_pool(name="ps_uv", bufs=5, space="PSUM") as ps_uv,
                tc.tile_pool(name="ps_o", bufs=3, space="PSUM") as ps_o,
                tc.tile_pool(name="wkb", bufs=4) as wb,
            ):
                prev = None  # deferred (pairs, h, i) consumed one step later
                for i in range(n_tiles + 1):
                    if i < n_tiles:
                        # --- y chain for tile i (front of DVE stream) ---
                        wc = wbl.tile([P, D], bf16, tag="wc", bufs=3)
                        nc.sync.dma_start(wc[:], wcum_dram[i])
                        kc = wbl.tile([P, D], bf16, tag="kc", bufs=3)
                        nc.sync.dma_start(kc[:], kvcum_dram[i])
                        sgq = wbl.tile([P, D], bf16, tag="sgq", bufs=3)
                        nc.sync.dma_start(sgq[:], sigq_dram[i])
                        sig = wb.tile([P, D], bf16, tag="sig")
                        nc.scalar.activation(sig[:], sgq[:], AF.Sigmoid)

                        twc = wb.tile([P, D], bf16, tag="twc")
                        nc.vector.tensor_add(twc[:], wc[:], cwb[:])
                        rec = wb.tile([P, D], bf16, tag="rec")
                        with nc.allow_low_precision(reason="y denominators are bf16 anyway"):
                            nc.vector.reciprocal(rec[:], twc[:])
                        tkc = wb.tile([P, D], bf16, tag="tkc")
                        nc.vector.tensor_add(tkc[:], kc[:], ckb[:])
                        yt = wb.tile([P, D], bf16, tag="yt")
                        nc.vector.tensor_mul(yt[:], tkc[:], rec[:])
                        y2 = wb.tile([P, D], bf16, tag="y2")
                        nc.vector.tensor_mul(y2[:], yt[:], sig[:])
                        y2T = wb.tile([P, 8, P], bf16, tag="y2T")
                        nc.sync.dma_start_transpose(y2T[:], y2[:])
                        if use_fp8:
                            y2T8 = wb.tile([P, 8, P], f8, tag="y2T8")
                            nc.scalar.copy(y2T8[:], y2T[:])
                        else:
                            y2T8 = y2T

                    # --- previous tile's silu/h consumption (frees uv psums) ---
                    if prev is not None:
                        pairs_p, h_p, ip = prev
                        for j, (pu, pg) in enumerate(pairs_p):
                            js = slice(j * H, (j + 1) * H)
                            sg = wb.tile([P, H], fp32, tag="sg", name=f"sg{j}")
                            nc.scalar.activation(sg[:], pg[:], AF.Sigmoid)
                            sl = wb.tile([P, H], fp32, tag="sl", name=f"sl{j}")
                            nc.vector.tensor_mul(sl[:], sg[:], pg[:])
                            nc.vector.tensor_mul(h_p[:, js], sl[:], pu[:])
                        hT = wb.tile([P, 8, P], bf16, tag="hT")
                        nc.sync.dma_start_transpose(hT[:], h_p[:])
                        if use_fp8:
                            hT8 = wb.tile([P, 8, P], f8, tag="hT8")
                            nc.scalar.copy(hT8[:], hT[:])
                        else:
                            hT8 = hT

                    # --- PE: swiglu mms for tile i ---
                    if i < n_tiles:
                        h = wb.tile([P, D], bf16, tag="h")
                        pairs = []
                        for j in range(2):
                            pu = ps_uv.tile([P, H], fp32, tag="uv", name=f"uv_u{j}")
                            pg = ps_uv.tile([P, H], fp32, tag="uv", name=f"uv_g{j}")
                            nk = 4 if use_fp8 else 8
                            for m in range(nk):
                                ms = slice(2 * m, 2 * m + 2) if use_fp8 else m
                                nc.tensor.matmul(
                                    pu[:], lhsT=y2T8[:, ms, :],
                                    rhs=wsw_sb[:, ms, j * H : (j + 1) * H],
                                    start=(m == 0), stop=(m == nk - 1),
                                    perf_mode=DR,
                                )
                                nc.tensor.matmul(
                                    pg[:], lhsT=y2T8[:, ms, :],
                                    rhs=wsw_sb[:, ms, (2 + j) * H : (3 + j) * H],
                                    start=(m == 0), stop=(m == nk - 1),
                                    perf_mode=DR,
                                )
                            pairs.append((pu, pg))

                    # --- PE: out mms for the previous tile ---
                    if prev is not None:
                        op = [
                            ps_o.tile([P, H], fp32, tag="op", name=f"op{n}")
                            for n in range(2)
                        ]
                        nk = 4 if use_fp8 else 8
                        for m in range(nk):
                            ms = slice(2 * m, 2 * m + 2) if use_fp8 else m
                            for n in range(2):
                                nc.tensor.matmul(
                                    op[n][:], lhsT=hT8[:, ms, :],
                                    rhs=wout_sb[:, ms, n * H : (n + 1) * H],
                                    start=(m == 0), stop=(m == nk - 1),
                                    perf_mode=DR,
                                )
                        xt2 = wb.tile([P, D], fp32, tag="xt2")
                        nc.sync.dma_start(xt2[:], x_t[ip])
                        for n in range(2):
                            ns = slice(n * H, (n + 1) * H)
                            nc.vector.tensor_add(xt2[:, ns], xt2[:, ns], op[n][:])
                        nc.sync.dma_start(out_t[ip], xt2[:])

                    if i < n_tiles:
                        prev = (pairs, h, i)

    nc.compile()
    return nc


def _host_inputs(x, w_qkv, w_swiglu, w_out, use_fp8=True):
    bf = ml_dtypes.bfloat16
    f8 = ml_dtypes.float8_e4m3fn if use_fp8 else bf
    wqkvT = np.ascontiguousarray(w_qkv.T).astype(f8)
    wswT = np.ascontiguousarray(w_swiglu.T).astype(f8)
    woutT = np.ascontiguousarray(w_out.T).astype(f8)
    tri = np.triu(np.ones((P, P), np.float32)).astype(bf)
    in_maps = []
    for c in range(N_CORES):
        b, h = c // 2, c % 2
        in_maps.append(
            {
                "x": np.ascontiguousarray(
                    x[b, h * CHUNK : (h + 1) * CHUNK, :]
                ).astype(np.float32),
                "wqkvT": wqkvT,
                "wswT": wswT,
                "woutT": woutT,
                "triT": tri,
                "cmask": np.full((2, 1), float(h), np.float32),
            }
        )
    return in_maps


def kernel(x, w_qkv, w_swiglu, w_out, trace=False):
    from concourse.bass_utils import run_bass_kernel_spmd

    x = np.asarray(x, dtype=np.float32)
    w_qkv = np.asarray(w_qkv, dtype=np.float32)
    w_swiglu = np.asarray(w_swiglu, dtype=np.float32)
    w_out = np.asarray(w_out, dtype=np.float32)

    key = "full"
    if key not in _nc_cache:
        _nc_cache[key] = build_nc(NT_FULL, N_CORES, use_collective=True, use_fp8=USE_FP8)
    nc = _nc_cache[key]

    in_maps = _host_inputs(x, w_qkv, w_swiglu, w_out, use_fp8=USE_FP8)
    res = run_bass_kernel_spmd(
        nc, in_maps, core_ids=list(range(N_CORES)), trace=trace
    )
    out = np.empty((B_FULL, T_FULL, D), np.float32)
    for c in range(N_CORES):
        b, h = c // 2, c % 2
        out[b, h * CHUNK : (h + 1) * CHUNK, :] = res.results[c]["out"]
    kernel.last_result = res
    return out

